# revision 5
# baseline (speedup 1.0000x reference)
"""Pairwise cosine-similarity kernel for Trainium2 (8 NeuronCores, SPMD).

Computes out = 16 * normalize(x1) @ normalize(x2).T for x1, x2 [8192, 512] f32.

Sharding: x1 rows are split across the 8 cores (1024 rows each); x2 is
replicated. Each core computes its [1024, 8192] slice of the output; the host
concatenates the slices.

Host-side prep is layout/dtype only: inputs are cast to bf16 and x2 is shipped
pre-transposed ([512, 8192]) so the big operand needs no on-device
transposition. The device writes the output in bf16 and the host widens it to
f32 (exact). All FLOPs (norms, normalization, GEMM, scaling) run on device:

  1. x1 (bf16, natural): fused Square+row-sum on ScalarE -> sqrt -> reciprocal
     -> x1n = x1 * (16/n1) via per-partition tensor_scalar, then PE-transpose
     (bf16 matmul vs. identity) into x1T [128, 4k, 512] per 512-row group.
  2. x2 norms directly from the transposed operand, per 1024-wide column
     group: Square on ScalarE, pairwise sums over the 4 K-chunks on GpSimd,
     then ones.T @ ssum on the PE -- one matmul both reduces over the
     partition (K) dim and broadcasts the result to all 128 partitions.
     sqrt (ScalarE) + reciprocal (DVE) -> inv [128, 1024] bf16, and one DVE
     tensor_tensor scales all 4 K-chunks of the column group in place.
  3. Main GEMM: out_tile[128, 1024] += x1T.T @ x2T over 4 K-chunks (bf16,
     f32 PSUM), PSUM->SBUF bf16 copies split across DVE/ACT, DMA out.

DMA: input loads all issue up front on the Scalar HWDGE ring; output stores
go on the Sync HWDGE ring. Total HBM traffic per core is 9 MB in + 16 MB out.
"""

import sys

for _p in ("/root/.axon_site/_ro/trn_rl_repo", "/opt/trn_rl_repo"):
    if _p not in sys.path:
        sys.path.append(_p)

import ml_dtypes
import numpy as np

import concourse.bass as bass
import concourse.tile as tile
from concourse import bacc, mybir
from concourse.bass_utils import run_bass_kernel_spmd
from concourse.masks import make_identity

F32 = mybir.dt.float32
BF16 = mybir.dt.bfloat16
P = 128
SCALE = 16.0
EPS = 1e-8

N_CORES = 8
N1 = 8192  # x1 rows (total)
N2 = 8192  # x2 rows
D = 512  # feature dim
KC = D // P  # K-chunks of the contraction dim

_PROGRAM_CACHE = {}


def build_program(n1_local=N1 // N_CORES, n2=N2, d=D, cg_width=1024):
    """Build the SPMD program one core runs. Returns the compiled Bacc.

    DRAM inputs: x1 [n1_local, d] bf16 (natural), x2t [d, n2] bf16
    (pre-transposed). DRAM output: out [n1_local, n2] bf16.
    """
    kc = d // P
    m_tiles = n1_local // P  # 8 output row-tiles
    n_cgs = n2 // cg_width  # 8 output column groups
    nch = cg_width // 512  # 512-wide PSUM chunks per column group
    n_gs = n1_local // 512  # 2 x1 row groups (4 row-tiles each)

    nc = bacc.Bacc("TRN2", target_bir_lowering=False, debug=False,
                   num_devices=N_CORES)
    x1 = nc.dram_tensor("x1", [n1_local, d], BF16, kind="ExternalInput")
    x2t = nc.dram_tensor("x2t", [d, n2], BF16, kind="ExternalInput")
    out = nc.dram_tensor("out", [n1_local, n2], BF16, kind="ExternalOutput")

    with tile.TileContext(nc) as tc:
        with (
            tc.tile_pool(name="const", bufs=1) as const,
            tc.tile_pool(name="ld", bufs=2) as ld,
            tc.tile_pool(name="xt", bufs=1) as xt,
            tc.tile_pool(name="sq", bufs=2) as sqp,
            tc.tile_pool(name="tmp", bufs=4) as tmp,
            tc.tile_pool(name="stat", bufs=4) as stat,
            tc.tile_pool(name="nrm", bufs=2) as nrmp,
            tc.tile_pool(name="inv", bufs=3) as invp,
            tc.tile_pool(name="outs", bufs=4) as outs,
            tc.tile_pool(name="pso", bufs=3, space="PSUM") as pso,
            tc.tile_pool(name="psa", bufs=1, space="PSUM") as psa,
        ):
            ident_b = const.tile([P, P], BF16)
            make_identity(nc, ident_b)
            ones_b = const.tile([P, P], BF16)
            nc.gpsimd.memset(ones_b[:], 1.0)

            # x1 natural rows, grouped: row = g*512 + j*128 + p
            x1r = x1.ap().rearrange("(g j p) e -> g p j e", j=4, p=P)
            # x2t rows grouped by K-chunk: row = k*128 + p
            x2r = x2t.ap().rearrange("(k p) n -> p k n", p=P)

            x1T = [xt.tile([P, kc, 512], BF16, tag=f"x1T_{g}",
                           name=f"x1T_{g}") for g in range(n_gs)]
            x2T = [xt.tile([P, kc, cg_width], BF16, tag=f"x2T_{cg}",
                           name=f"x2T_{cg}") for cg in range(n_cgs)]

            # ---- all input DMAs up front (scalar HWDGE ring) -------------
            # cg0 first so its norm/scale chain starts as early as possible.
            def load_cg(cg):
                nc.scalar.dma_start(
                    x2T[cg][:],
                    x2r[:, :, cg * cg_width : (cg + 1) * cg_width],
                )

            load_cg(0)
            ld_x1 = []
            for g in range(n_gs):
                ld_t = ld.tile([P, 4, d], BF16, tag="ld", name=f"ld_{g}")
                nc.scalar.dma_start(ld_t[:], x1r[g])
                ld_x1.append(ld_t)
            for cg in range(1, n_cgs):
                load_cg(cg)

            # ---- x1: stats -> normalize (bf16) -> PE transpose -----------
            for g in range(n_gs):
                ld_t = ld_x1[g]
                ssq = stat.tile([P, 4], F32, tag="ssq")
                for j in range(4):
                    sq_t = tmp.tile([P, d], BF16, tag="sq1")
                    nc.scalar.activation(
                        sq_t[:], ld_t[:, j],
                        mybir.ActivationFunctionType.Square,
                        accum_out=ssq[:, j : j + 1],
                    )
                # nrm1 = sqrt(ssq)/SCALE; inv1 = SCALE/sqrt(ssq)
                nrm1 = stat.tile([P, 4], F32, tag="nrm1")
                nc.scalar.activation(
                    nrm1[:], ssq[:], mybir.ActivationFunctionType.Sqrt,
                    scale=1.0 / (SCALE * SCALE),
                )
                inv1 = stat.tile([P, 4], F32, tag="inv1")
                nc.vector.reciprocal(inv1[:], nrm1[:])
                x1n = tmp.tile([P, 4, d], BF16, tag="x1n")
                for j in range(4):
                    nc.vector.tensor_scalar_mul(
                        x1n[:, j], ld_t[:, j], inv1[:, j : j + 1]
                    )
                for k in range(kc):
                    ps_t = psa.tile([P, cg_width], F32, tag="psa",
                                    name=f"psx1_{g}_{k}")
                    for j in range(4):
                        nc.tensor.matmul(
                            ps_t[:, j * P : (j + 1) * P],
                            lhsT=x1n[:, j, k * P : (k + 1) * P],
                            rhs=ident_b[:],
                            start=True, stop=True,
                        )
                    dst = x1T[g][:, k, :]
                    if k % 2 == 0:
                        nc.vector.tensor_copy(dst, ps_t[:, :512])
                    else:
                        nc.scalar.copy(dst, ps_t[:, :512])

            # ---- x2 per column group: norms from x2T, scale in place -----
            def prep_cg(cg):
                # squares + pairwise K-chunk sums on GpSimd: it is otherwise
                # idle, so this never queues behind GEMM-drain copies.
                xc = x2T[cg]
                sq_t = sqp.tile([P, kc, cg_width], BF16, tag="sq2",
                                name=f"sq2_{cg}")
                nc.gpsimd.tensor_mul(sq_t[:], xc[:], xc[:])
                s01 = tmp.tile([P, cg_width], BF16, tag="s01")
                s23 = tmp.tile([P, cg_width], BF16, tag="s23")
                ssum = tmp.tile([P, cg_width], BF16, tag="ssum")
                nc.gpsimd.tensor_add(s01[:], sq_t[:, 0], sq_t[:, 1])
                nc.gpsimd.tensor_add(s23[:], sq_t[:, 2], sq_t[:, 3])
                nc.gpsimd.tensor_add(ssum[:], s01[:], s23[:])
                # ones.T @ ssum: reduces over partitions AND broadcasts the
                # column sums to all 128 partitions in one pass.
                ps_s = psa.tile([P, cg_width], F32, tag="psa",
                                name=f"psn_{cg}")
                for c in range(nch):
                    nc.tensor.matmul(
                        ps_s[:, c * 512 : (c + 1) * 512],
                        lhsT=ones_b[:],
                        rhs=ssum[:, c * 512 : (c + 1) * 512],
                        start=True, stop=True,
                    )
                nrm = nrmp.tile([P, cg_width], F32, tag="nrm",
                                name=f"nrm_{cg}")
                nc.scalar.activation(
                    nrm[:], ps_s[:], mybir.ActivationFunctionType.Sqrt
                )
                inv = invp.tile([P, cg_width], BF16, tag="inv",
                                name=f"inv_{cg}")
                with nc.allow_low_precision(
                    reason="row norms are ~sqrt(D); bf16 inverse norm is "
                    "plenty for the 2e-2 gate"
                ):
                    nc.vector.reciprocal(inv[:], nrm[:])
                nc.vector.tensor_mul(
                    xc[:], xc[:], inv[:, None, :].to_broadcast((P, kc, cg_width))
                )

            def gemm_m(cg, m):
                lhs = x1T[m // 4]
                mm = m % 4
                ps = pso.tile([P, cg_width], F32, tag="ps",
                              name=f"ps_{cg}_{m}")
                for k in range(kc):
                    for c in range(nch):
                        nc.tensor.matmul(
                            ps[:, c * 512 : (c + 1) * 512],
                            lhsT=lhs[:, k, mm * P : (mm + 1) * P],
                            rhs=x2T[cg][:, k, c * 512 : (c + 1) * 512],
                            start=(k == 0), stop=(k == kc - 1),
                        )
                ot = outs.tile([P, cg_width], BF16, tag="ot",
                               name=f"ot_{cg}_{m}")
                nc.vector.tensor_copy(ot[:, :512], ps[:, :512])
                nc.scalar.copy(ot[:, 512:], ps[:, 512:])
                nc.sync.dma_start(
                    out[m * P : (m + 1) * P,
                        cg * cg_width : (cg + 1) * cg_width],
                    ot[:],
                )

            prep_cg(0)
            prep_cg(1)
            for cg in range(n_cgs):
                for m in range(m_tiles):
                    gemm_m(cg, m)
                    # emit the cg+2 norm pipeline early in this gemm so its
                    # PE/ACT/DVE slots clear long before gemm(cg+2) starts
                    if m == 1 and cg + 2 < n_cgs:
                        prep_cg(cg + 2)

    nc.compile()
    return nc


def _get_program():
    key = "default"
    if key not in _PROGRAM_CACHE:
        _PROGRAM_CACHE[key] = build_program()
    return _PROGRAM_CACHE[key]


def make_in_maps(x1: np.ndarray, x2: np.ndarray) -> list:
    x1 = np.asarray(x1, dtype=np.float32)
    x2 = np.asarray(x2, dtype=np.float32)
    assert x1.shape == (N1, D) and x2.shape == (N2, D), (x1.shape, x2.shape)
    x1_b = x1.astype(ml_dtypes.bfloat16)
    x2t_b = np.ascontiguousarray(x2.astype(ml_dtypes.bfloat16).T)
    rows = N1 // N_CORES
    return [
        {
            "x1": np.ascontiguousarray(x1_b[c * rows : (c + 1) * rows]),
            "x2t": x2t_b,
        }
        for c in range(N_CORES)
    ]


def kernel(x1: np.ndarray, x2: np.ndarray) -> np.ndarray:
    nc = _get_program()
    in_maps = make_in_maps(x1, x2)
    res = run_bass_kernel_spmd(nc, in_maps, core_ids=list(range(N_CORES)))
    return np.concatenate(
        [res.results[c]["out"] for c in range(N_CORES)], axis=0
    ).astype(np.float32)


if __name__ == "__main__":
    rng = np.random.default_rng(0)
    a = rng.standard_normal((N1, D), dtype=np.float32)
    b = rng.standard_normal((N2, D), dtype=np.float32)
    got = kernel(a, b)
    n1 = np.maximum(np.linalg.norm(a, axis=-1, keepdims=True), EPS)
    n2 = np.maximum(np.linalg.norm(b, axis=-1, keepdims=True), EPS)
    want = SCALE * (a / n1) @ (b / n2).T
    err = np.abs(got - want)
    rel = np.linalg.norm(got - want) / np.linalg.norm(want)
    print(f"max abs err: {err.max():.3e}  rel: {rel:.3e}")


# revision 6
# speedup vs baseline: 1.5553x; 1.5553x over previous
"""Pairwise cosine-similarity kernel for Trainium2 (8 NeuronCores, SPMD).

Computes out = 16 * normalize(x1) @ normalize(x2).T for x1, x2 [8192, 512] f32.

Sharding: x1 rows are split across the 8 cores (1024 rows each); x2 is
replicated. Each core computes its [1024, 8192] slice of the output; the host
concatenates the slices.

Host-side prep is layout/dtype only: inputs are cast to bf16 and x2 is shipped
pre-transposed and column-group-blocked ([8, 128, 4, 1024]) so each 1MB load
is 128 contiguous 8KB descriptors. The device writes the output in bf16 and
the host widens it to f32 (exact). All FLOPs run on device:

  1. x1 (bf16, natural): fused Square+row-sum on ScalarE -> sqrt -> DVE
     reciprocal -> x1n = x1 * (16/n1) via ScalarE copy-with-per-partition-
     scale, then PE-transpose (bf16 matmul vs. identity) into x1T.
  2. x2 column norms per 1024-wide column group, off the GEMM's critical
     path: Square (ScalarE) -> pairwise K-chunk adds (DVE) -> ones.T @ ssum
     on the PE (one matmul reduces over the partition dim AND broadcasts to
     all 128 partitions) -> sqrt (ScalarE) -> reciprocal_approx_fast (DVE)
     -> inv [128, 1024] f32. x2T itself stays raw.
  3. Main GEMM on raw x2T: out_tile[128, 1024] += x1T.T @ x2T over 4
     K-chunks (bf16, f32 PSUM). The PSUM->SBUF drain is a DVE tensor_mul by
     inv (the column inverse norms), so normalization costs nothing extra.

DMA: input loads issue up front on the Scalar HWDGE ring; output stores go on
the Sync HWDGE ring. HBM traffic per core: 9 MB in + 16 MB out.
"""

import sys

for _p in ("/root/.axon_site/_ro/trn_rl_repo", "/opt/trn_rl_repo"):
    if _p not in sys.path:
        sys.path.append(_p)

import ml_dtypes
import numpy as np

import concourse.bass as bass
import concourse.tile as tile
from concourse import bacc, mybir
from concourse.bass_utils import run_bass_kernel_spmd
from concourse.masks import make_identity

F32 = mybir.dt.float32
BF16 = mybir.dt.bfloat16
P = 128
SCALE = 16.0
EPS = 1e-8

N_CORES = 8
N1 = 8192  # x1 rows (total)
N2 = 8192  # x2 rows
D = 512  # feature dim
KC = D // P  # K-chunks of the contraction dim
CGW = 1024  # output column-group width

_PROGRAM_CACHE = {}


def build_program(n1_local=N1 // N_CORES, n2=N2, d=D, cg_width=CGW):
    """Build the SPMD program one core runs. Returns the compiled Bacc."""
    kc = d // P
    m_tiles = n1_local // P  # 8 output row-tiles
    n_cgs = n2 // cg_width  # 8 output column groups
    nch = cg_width // 512  # 512-wide PSUM chunks per column group
    n_gs = n1_local // 512  # 2 x1 row groups (4 row-tiles each)

    nc = bacc.Bacc("TRN2", target_bir_lowering=False, debug=False,
                   num_devices=N_CORES)
    x1 = nc.dram_tensor("x1", [n1_local, d], BF16, kind="ExternalInput")
    x2t = nc.dram_tensor("x2t", [n_cgs, P, kc, cg_width], BF16,
                         kind="ExternalInput")
    out = nc.dram_tensor("out", [n1_local, n2], BF16, kind="ExternalOutput")

    with tile.TileContext(nc) as tc:
        with (
            tc.tile_pool(name="const", bufs=1) as const,
            tc.tile_pool(name="ld", bufs=2) as ld,
            tc.tile_pool(name="xt", bufs=1) as xt,
            tc.tile_pool(name="sq", bufs=2) as sqp,
            tc.tile_pool(name="tmp", bufs=4) as tmp,
            tc.tile_pool(name="stat", bufs=4) as stat,
            tc.tile_pool(name="nrm", bufs=2) as nrmp,
            tc.tile_pool(name="inv", bufs=3) as invp,
            tc.tile_pool(name="outs", bufs=4) as outs,
            tc.tile_pool(name="pso", bufs=3, space="PSUM") as pso,
            tc.tile_pool(name="psa", bufs=2, space="PSUM") as psa,
        ):
            ident_b = const.tile([P, P], BF16)
            make_identity(nc, ident_b)
            ones_b = const.tile([P, P], BF16)
            nc.gpsimd.memset(ones_b[:], 1.0)

            # x1 natural rows, grouped: row = g*512 + j*128 + p
            x1r = x1.ap().rearrange("(g j p) e -> g p j e", j=4, p=P)

            x1T = [xt.tile([P, kc, 512], BF16, tag=f"x1T_{g}",
                           name=f"x1T_{g}") for g in range(n_gs)]
            x2T = [xt.tile([P, kc, cg_width], BF16, tag=f"x2T_{cg}",
                           name=f"x2T_{cg}") for cg in range(n_cgs)]
            invs = [None] * n_cgs

            # ---- input DMAs up front (scalar HWDGE ring) -----------------
            ld_x1 = []
            ld_t = ld.tile([P, 4, d], BF16, tag="ld", name="ld_0")
            nc.scalar.dma_start(ld_t[:], x1r[0])
            ld_x1.append(ld_t)
            nc.scalar.dma_start(x2T[0][:], x2t.ap()[0])
            ld_t = ld.tile([P, 4, d], BF16, tag="ld", name="ld_1")
            nc.scalar.dma_start(ld_t[:], x1r[1])
            ld_x1.append(ld_t)
            for cg in range(1, n_cgs):
                nc.scalar.dma_start(x2T[cg][:], x2t.ap()[cg])

            # ---- x1: stats -> normalize (bf16) -> PE transpose -----------
            for g in range(n_gs):
                ld_t = ld_x1[g]
                ssq = stat.tile([P, 4], F32, tag="ssq")
                for j in range(4):
                    sq_t = tmp.tile([P, d], BF16, tag="sq1")
                    nc.scalar.activation(
                        sq_t[:], ld_t[:, j],
                        mybir.ActivationFunctionType.Square,
                        accum_out=ssq[:, j : j + 1],
                    )
                # nrm1 = sqrt(ssq)/SCALE; inv1 = SCALE/sqrt(ssq)
                nrm1 = stat.tile([P, 4], F32, tag="nrm1")
                nc.scalar.activation(
                    nrm1[:], ssq[:], mybir.ActivationFunctionType.Sqrt,
                    scale=1.0 / (SCALE * SCALE),
                )
                inv1 = stat.tile([P, 4], F32, tag="inv1")
                nc.vector.reciprocal(inv1[:], nrm1[:])
                x1n = tmp.tile([P, 4, d], BF16, tag="x1n")
                for j in range(4):
                    nc.scalar.activation(
                        x1n[:, j], ld_t[:, j],
                        mybir.ActivationFunctionType.Copy,
                        scale=inv1[:, j : j + 1],
                    )
                for k in range(kc):
                    ps_t = psa.tile([P, 512], F32, tag="psa",
                                    name=f"psx1_{g}_{k}")
                    for j in range(4):
                        nc.tensor.matmul(
                            ps_t[:, j * P : (j + 1) * P],
                            lhsT=x1n[:, j, k * P : (k + 1) * P],
                            rhs=ident_b[:],
                            start=True, stop=True,
                        )
                    dst = x1T[g][:, k, :]
                    if k % 2 == 0:
                        nc.vector.tensor_copy(dst, ps_t[:])
                    else:
                        nc.scalar.copy(dst, ps_t[:])

            # ---- x2 column norms (pre: squares + K-chunk sums) -----------
            def prep_pre(cg):
                xc = x2T[cg]
                sq_t = sqp.tile([P, kc, cg_width], BF16, tag="sq2",
                                name=f"sq2_{cg}")
                nc.scalar.activation(
                    sq_t[:], xc[:], mybir.ActivationFunctionType.Square
                )
                s01 = tmp.tile([P, cg_width], BF16, tag="s01")
                s23 = tmp.tile([P, cg_width], BF16, tag="s23")
                ssum = tmp.tile([P, cg_width], BF16, tag="ssum",
                                name=f"ssum_{cg}")
                nc.vector.tensor_add(s01[:], sq_t[:, 0], sq_t[:, 1])
                nc.vector.tensor_add(s23[:], sq_t[:, 2], sq_t[:, 3])
                nc.vector.tensor_add(ssum[:], s01[:], s23[:])
                return ssum

            # ---- reduce over partitions + broadcast, sqrt, reciprocal ----
            def prep_post(cg, ssum):
                inv = invp.tile([P, cg_width], F32, tag="inv",
                                name=f"inv_{cg}")
                for c in range(nch):
                    cs = slice(c * 512, (c + 1) * 512)
                    ps_s = psa.tile([P, 512], F32, tag="psa",
                                    name=f"psn_{cg}_{c}")
                    nc.tensor.matmul(ps_s[:], lhsT=ones_b[:], rhs=ssum[:, cs],
                                     start=True, stop=True)
                    nrm = nrmp.tile([P, 512], F32, tag="nrm",
                                    name=f"nrm_{cg}_{c}")
                    nc.scalar.activation(
                        nrm[:], ps_s[:], mybir.ActivationFunctionType.Sqrt
                    )
                    nc.vector.reciprocal_approx_fast(inv[:, cs], nrm[:])
                invs[cg] = inv

            def gemm_m(cg, m):
                lhs = x1T[m // 4]
                mm = m % 4
                ps = pso.tile([P, cg_width], F32, tag="ps",
                              name=f"ps_{cg}_{m}")
                for k in range(kc):
                    for c in range(nch):
                        nc.tensor.matmul(
                            ps[:, c * 512 : (c + 1) * 512],
                            lhsT=lhs[:, k, mm * P : (mm + 1) * P],
                            rhs=x2T[cg][:, k, c * 512 : (c + 1) * 512],
                            start=(k == 0), stop=(k == kc - 1),
                        )
                if m == 0:
                    prep_post(cg, ssums[cg])
                ot = outs.tile([P, cg_width], BF16, tag="ot",
                               name=f"ot_{cg}_{m}")
                # normalize x2 columns during the PSUM drain
                nc.vector.tensor_mul(ot[:, :512], ps[:, :512],
                                     invs[cg][:, :512])
                nc.vector.tensor_mul(ot[:, 512:], ps[:, 512:],
                                     invs[cg][:, 512:])
                nc.sync.dma_start(
                    out[m * P : (m + 1) * P,
                        cg * cg_width : (cg + 1) * cg_width],
                    ot[:],
                )

            ssums = [None] * n_cgs
            ssums[0] = prep_pre(0)
            ssums[1] = prep_pre(1)
            for cg in range(n_cgs):
                for m in range(m_tiles):
                    gemm_m(cg, m)
                    if m == 1 and cg + 2 < n_cgs:
                        ssums[cg + 2] = prep_pre(cg + 2)

    nc.compile()
    return nc


def _get_program():
    key = "default"
    if key not in _PROGRAM_CACHE:
        _PROGRAM_CACHE[key] = build_program()
    return _PROGRAM_CACHE[key]


def make_in_maps(x1: np.ndarray, x2: np.ndarray) -> list:
    x1 = np.asarray(x1, dtype=np.float32)
    x2 = np.asarray(x2, dtype=np.float32)
    assert x1.shape == (N1, D) and x2.shape == (N2, D), (x1.shape, x2.shape)
    x1_b = x1.astype(ml_dtypes.bfloat16)
    # [512, 8192] -> cg-blocked [8, 128, 4, 1024]: x2t[k*128+p, cg*1024+c]
    # lands at x2tb[cg, p, k, c], so each column group is one contiguous
    # 8KB-per-partition DMA.
    x2t = x2.astype(ml_dtypes.bfloat16).T
    x2tb = np.ascontiguousarray(
        x2t.reshape(KC, P, N2 // CGW, CGW).transpose(2, 1, 0, 3)
    )
    rows = N1 // N_CORES
    return [
        {
            "x1": np.ascontiguousarray(x1_b[c * rows : (c + 1) * rows]),
            "x2t": x2tb,
        }
        for c in range(N_CORES)
    ]


def kernel(x1: np.ndarray, x2: np.ndarray) -> np.ndarray:
    nc = _get_program()
    in_maps = make_in_maps(x1, x2)
    res = run_bass_kernel_spmd(nc, in_maps, core_ids=list(range(N_CORES)))
    return np.concatenate(
        [res.results[c]["out"] for c in range(N_CORES)], axis=0
    ).astype(np.float32)


if __name__ == "__main__":
    rng = np.random.default_rng(0)
    a = rng.standard_normal((N1, D), dtype=np.float32)
    b = rng.standard_normal((N2, D), dtype=np.float32)
    got = kernel(a, b)
    n1 = np.maximum(np.linalg.norm(a, axis=-1, keepdims=True), EPS)
    n2 = np.maximum(np.linalg.norm(b, axis=-1, keepdims=True), EPS)
    want = SCALE * (a / n1) @ (b / n2).T
    err = np.abs(got - want)
    rel = np.linalg.norm(got - want) / np.linalg.norm(want)
    print(f"max abs err: {err.max():.3e}  rel: {rel:.3e}")
